# revision 1
# baseline (speedup 1.0000x reference)
"""Causal multi-head attention block on 8 Trainium2 NeuronCores.

Sharding: data parallel over batch (B == 8 == n_cores); each core runs one
batch element with full weights, no collectives.

Design (per core):
  - host pre-transposes x and pre-casts x^T / w_attn / w_proj to bf16; all
    matmuls run bf16 operands with f32 PSUM accumulation (rel err ~3.5e-3).
  - inputs stream on both HWDGE queues (sync + scalar) in consumption order.
  - B1: v in natural [T, (H, 64+1)] layout with a ones-column so the PV
    matmul also produces softmax denominators.  B2: q^T/k^T tiles [n, T],
    emitted per head-pair and interleaved with attention so the in-order
    tensor-engine stream always has fill work.
  - attention per (head, 512-wide i-slab): for each j-tile one QK matmul
    over the causal i-suffix (S^T layout, [j, i]) and one PV matmul
    accumulating into a [65, 512] PSUM tile; two j-tiles share one
    [128, 1024] PSUM pair-tile so one exp ACT covers both (gap-free
    packing).  Two heads are interleaved at pair-unit granularity.
  - causal diagonal masked by a bf16 triangular multiply on the vector
    engine; softmax denominators via one reciprocal_approx_fast per
    (head, slab) + gpsimd partition broadcast.
  - dummy PE warm-up transposes at t=0 and filler matmuls in the tail keep
    the HAM activity monitor from halving the PE clock during
    scalar-bound stretches.
"""

import numpy as np
import ml_dtypes
from contextlib import ExitStack

import concourse.bass as bass
import concourse.mybir as mybir
import concourse.tile as tile
from concourse import bacc
from concourse.bass_utils import run_bass_kernel_spmd
from concourse.masks import make_identity

F32 = mybir.dt.float32
BF16 = mybir.dt.bfloat16
AF = mybir.ActivationFunctionType

B, T, C = 8, 1024, 768
H, HS = 12, 64
KT = C // 128            # 6 contraction tiles
MT = T // 128            # 8 row tiles (also j tiles)
SCALE = 1.0 / np.sqrt(HS)

N_CORES = 8


def build_program():
    nc = bacc.Bacc("TRN2", target_bir_lowering=False, debug=False)

    xt = nc.dram_tensor("xt", [C, T], BF16, kind="ExternalInput")
    w16 = nc.dram_tensor("w16", [C, 3 * C], BF16, kind="ExternalInput")
    b_attn = nc.dram_tensor("b_attn", [3 * C], F32, kind="ExternalInput")
    wp16d = nc.dram_tensor("wp16", [C, C], BF16, kind="ExternalInput")
    b_proj = nc.dram_tensor("b_proj", [C], F32, kind="ExternalInput")
    out = nc.dram_tensor("out", [T, C], F32, kind="ExternalOutput")

    with tile.TileContext(nc) as tc, ExitStack() as ctx:
        consts = ctx.enter_context(tc.tile_pool(name="consts", bufs=1))
        big = ctx.enter_context(tc.tile_pool(name="big", bufs=1))
        io = ctx.enter_context(tc.tile_pool(name="io", bufs=2))
        pt_pool = ctx.enter_context(tc.tile_pool(name="pt", bufs=4))
        rl_pool = ctx.enter_context(tc.tile_pool(name="rl", bufs=3))
        ps_st = ctx.enter_context(tc.tile_pool(name="ps_st", bufs=2, space="PSUM"))
        ps_big = ctx.enter_context(tc.tile_pool(name="ps_big", bufs=2, space="PSUM"))
        ps_y = ctx.enter_context(tc.tile_pool(name="ps_y", bufs=2, space="PSUM"))

        # ---- constants ----
        ident = consts.tile([128, 128], F32, tag="ident")
        make_identity(nc, ident)
        ident16 = consts.tile([128, 128], BF16, tag="ident16")
        make_identity(nc, ident16)
        # tri16: 1.0 where f >= p (keep j <= i on an exact-diagonal block)
        tri16 = consts.tile([128, 128], BF16, tag="tri16")
        nc.gpsimd.memset(tri16, 1.0)
        nc.gpsimd.affine_select(
            out=tri16, in_=tri16, compare_op=mybir.AluOpType.is_ge,
            fill=0.0, base=0, pattern=[[1, 128]], channel_multiplier=-1,
        )
        battn_t = consts.tile([128, 18], F32, tag="battn_t")
        nc.sync.dma_start(out=battn_t, in_=b_attn[:].rearrange("(t p) -> p t", p=128))
        bv_b = consts.tile([128, C], F32, tag="bias_b")
        nc.sync.dma_start(
            out=bv_b,
            in_=bass.AP(tensor=b_attn[:].tensor, offset=2 * C, ap=[[0, 128], [1, C]]),
        )
        ones12 = consts.tile([128, H], BF16, tag="ones12")
        nc.gpsimd.memset(ones12, 1.0)

        # ---- PE warm-up: keep the HAM activity window busy during DMAs ----
        for i in range(64):
            pt = ps_big.tile([128, 512], F32, tag="mm")
            nc.tensor.transpose(pt[:, 0:128], ident, ident)

        # ---- input DMAs ----
        # arrival order matches consumption: xT, V columns (B1), Q then K
        # columns (B2), w_proj (D); split across the two HWDGE queues.
        xT = big.tile([128, KT, T], BF16, tag="xT")
        for kt in range(KT):
            eng = nc.scalar if kt % 2 == 0 else nc.sync
            eng.dma_start(out=xT[:, kt, :], in_=xt[kt * 128:(kt + 1) * 128, :])
        wr16 = big.tile([128, KT, 3 * C], BF16, tag="w16")
        w_dram = w16[:].rearrange("(t p) n -> p t n", p=128)
        nc.sync.dma_start(out=wr16[:, 0:3, 2 * C:3 * C], in_=w_dram[:, 0:3, 2 * C:3 * C])
        nc.scalar.dma_start(out=wr16[:, 3:6, 2 * C:3 * C], in_=w_dram[:, 3:6, 2 * C:3 * C])
        nc.scalar.dma_start(out=wr16[:, :, 0:C], in_=w_dram[:, :, 0:C])
        nc.sync.dma_start(out=wr16[:, :, C:2 * C], in_=w_dram[:, :, C:2 * C])
        wpr16 = big.tile([128, KT, C], BF16, tag="wp16")
        wp_dram = wp16d[:].rearrange("(t p) n -> p t n", p=128)
        nc.scalar.dma_start(out=wpr16, in_=wp_dram)

        # ---- phase B2 (qkT tiles, emitted per head-pair) ----
        qk16 = big.tile([128, 12, T], BF16, tag="qk16")

        def emit_b2_pair(pair):
            for nt in (pair, 6 + pair):
                for mc in range(2):
                    ps = ps_big.tile([128, 512], F32, tag="mm")
                    for kt in range(KT):
                        nc.tensor.matmul(
                            ps, wr16[:, kt, nt * 128:(nt + 1) * 128],
                            xT[:, kt, mc * 512:(mc + 1) * 512],
                            start=(kt == 0), stop=(kt == KT - 1),
                        )
                    nc.scalar.activation(
                        qk16[:, nt, mc * 512:(mc + 1) * 512], ps,
                        AF.Identity, bias=battn_t[:, nt:nt + 1], scale=1.0,
                    )

        # ---- phase B1: v_aug [T, H, 65] (natural layout + ones column) ----
        v_aug = big.tile([128, MT, H, HS + 1], BF16, tag="v_aug")
        for mt in range(MT):
            nc.vector.tensor_copy(
                v_aug[:, mt, :, HS:HS + 1].rearrange("p h o -> p (h o)"), ones12)
            for (n0, nsz) in [(0, 512), (512, 256)]:
                ps = ps_big.tile([128, 512], F32, tag="mm")
                for kt in range(KT):
                    nc.tensor.matmul(
                        ps[:, 0:nsz], xT[:, kt, mt * 128:(mt + 1) * 128],
                        wr16[:, kt, 2 * C + n0:2 * C + n0 + nsz],
                        start=(kt == 0), stop=(kt == KT - 1),
                    )
                h0, nh = n0 // HS, nsz // HS
                nc.vector.tensor_add(
                    v_aug[:, mt, h0:h0 + nh, 0:HS],
                    ps[:, 0:nsz].rearrange("p (h d) -> p h d", d=HS),
                    bv_b[:, n0:n0 + nsz].rearrange("p (h d) -> p h d", d=HS),
                )

        emit_b2_pair(0)

        # ---- phase C: attention per (head, 512-wide i-slab) ----
        yT16 = big.tile([128, KT, T], BF16, tag="yT16")

        def emit_attn_head_pair(h0):
            # two heads with independent dependency chains, interleaved at
            # pair-unit granularity so the scalar engine never starves
            heads = []
            for h in (h0, h0 + 1):
                nt_q, po = h // 2, 64 * (h % 2)
                heads.append((h, nt_q, po,
                              qk16[po:po + 64, nt_q, :],
                              qk16[po:po + 64, 6 + nt_q, :]))
            for slab in range(2):
                i0 = 512 * slab
                njt = 4 * (slab + 1)
                pairs = []
                for jp in range(njt // 2):
                    jtA, jtB = 2 * jp, 2 * jp + 1
                    nA = i0 + 512 - max(i0, jtA * 128)
                    nB = i0 + 512 - max(i0, jtB * 128)
                    offB = 512 if nA == 512 else nA
                    pairs.append((jtA, jtB, nA, nB, offB))

                yas = {}
                for h, nt_q, po, qT_h, kT_h in heads:
                    yas[h] = ps_y.tile([HS + 1, 512], F32, tag="ya",
                                       name=f"ya_{h}_{slab}")

                def emit_qk_pair(hh, pr):
                    h, nt_q, po, qT_h, kT_h = hh
                    jtA, jtB, nA, nB, offB = pr
                    st2 = ps_st.tile([128, 1024], F32, tag="st2")
                    for jt, off, n in ((jtA, 0, nA), (jtB, offB, nB)):
                        nc.tensor.matmul(
                            st2[:, off:off + n],
                            kT_h[:, jt * 128:(jt + 1) * 128],
                            qT_h[:, i0 + 512 - n:i0 + 512],
                            start=True, stop=True,
                        )
                    pt2 = pt_pool.tile([128, 1024], BF16, tag="ptile")
                    nc.scalar.activation(
                        pt2[:, 0:offB + nB], st2[:, 0:offB + nB],
                        AF.Exp, bias=0.0, scale=SCALE)
                    for jt, off in ((jtA, 0), (jtB, offB)):
                        if jt * 128 >= i0:  # diagonal: zero j > i on DVE
                            nc.vector.tensor_mul(
                                pt2[:, off:off + 128], pt2[:, off:off + 128],
                                tri16)
                    return (hh, pr, pt2)

                def emit_pv(item):
                    (h, nt_q, po, qT_h, kT_h), pr, pt2 = item
                    jtA, jtB, nA, nB, offB = pr
                    for jt, off, n in ((jtA, 0, nA), (jtB, offB, nB)):
                        nc.tensor.matmul(
                            yas[h][:, 512 - n:512], v_aug[:, jt, h, :],
                            pt2[:, off:off + n],
                            start=(jt == 0), stop=(jt == njt - 1),
                        )

                stage = []
                for pr in pairs:
                    for hh in heads:
                        stage.append(emit_qk_pair(hh, pr))
                        if len(stage) > 2:
                            emit_pv(stage.pop(0))
                while stage:
                    emit_pv(stage.pop(0))

                for h, nt_q, po, qT_h, kT_h in heads:
                    ya = yas[h]
                    lrow = rl_pool.tile([1, 512], F32, tag="lrow")
                    nc.vector.tensor_copy(lrow, ya[HS:HS + 1, :])
                    rl = rl_pool.tile([1, 512], F32, tag="rl")
                    nc.vector.reciprocal_approx_fast(rl, lrow)
                    rlb = rl_pool.tile([64, 512], F32, tag="rlb")
                    nc.gpsimd.partition_broadcast(rlb, rl)
                    nc.vector.tensor_mul(
                        yT16[po:po + 64, nt_q, i0:i0 + 512], ya[0:HS, :], rlb)
                if h0 >= 8:  # tail region; hold the HAM clock
                    for _ in range(10 if h0 >= 10 else 5):
                        fl = ps_big.tile([128, 512], F32, tag="mm")
                        nc.tensor.matmul(fl[:, 0:128], ident16, ident16,
                                         start=True, stop=True)

        # interleave: B2 pair p, then attention heads of pair p-1
        for p in range(1, 5):
            emit_b2_pair(p)
            emit_attn_head_pair(2 * (p - 1))
        emit_attn_head_pair(8)
        emit_b2_pair(5)
        emit_attn_head_pair(10)

        # ---- phase D: output projection (streamed per m-tile) ----
        bp_b = consts.tile([128, C], F32, tag="bias_b")  # reuses bv_b's slot
        nc.sync.dma_start(
            out=bp_b,
            in_=bass.AP(tensor=b_proj[:].tensor, offset=0, ap=[[0, 128], [1, C]]),
        )
        for mt in range(MT):
            out_sb = io.tile([128, C], F32, tag="io")
            for (c0, csz) in [(0, 512), (512, 256)]:
                ps = ps_big.tile([128, 512], F32, tag="mm")
                for nt in range(KT):
                    nc.tensor.matmul(
                        ps[:, 0:csz], yT16[:, nt, mt * 128:(mt + 1) * 128],
                        wpr16[:, nt, c0:c0 + csz],
                        start=(nt == 0), stop=(nt == KT - 1),
                    )
                nc.vector.tensor_add(
                    out_sb[:, c0:c0 + csz], ps[:, 0:csz], bp_b[:, c0:c0 + csz])
            eng = nc.scalar if mt % 2 == 0 else nc.sync
            eng.dma_start(
                out=out[mt * 128:(mt + 1) * 128, :], in_=out_sb)

    nc.compile()
    return nc


_CACHE = {}


def _get_program():
    if "nc" not in _CACHE:
        _CACHE["nc"] = build_program()
    return _CACHE["nc"]


def _prep_host_inputs(x, w_attn, b_attn, w_proj, b_proj):
    BF = ml_dtypes.bfloat16
    x = np.asarray(x, dtype=np.float32)
    xt16 = np.ascontiguousarray(np.transpose(x, (0, 2, 1))).astype(BF)
    w16 = np.ascontiguousarray(np.asarray(w_attn, np.float32)).astype(BF)
    wp16 = np.ascontiguousarray(np.asarray(w_proj, np.float32)).astype(BF)
    ba = np.asarray(b_attn, np.float32)
    bp = np.asarray(b_proj, np.float32)
    return [
        {
            "xt": np.ascontiguousarray(xt16[b]),
            "w16": w16,
            "b_attn": ba,
            "wp16": wp16,
            "b_proj": bp,
        }
        for b in range(B)
    ]


def kernel(x, w_attn, b_attn, w_proj, b_proj):
    nc = _get_program()
    in_maps = _prep_host_inputs(x, w_attn, b_attn, w_proj, b_proj)
    res = run_bass_kernel_spmd(nc, in_maps, list(range(N_CORES)))
    return np.stack([res.results[b]["out"] for b in range(B)], axis=0)

